# revision 35
# baseline (speedup 1.0000x reference)
"""AttnBlock (GroupNorm + single-head global attention + residual) on 8 trn2 cores.

Sharding: core c handles batch b = c//2, query-half h = c%2 (Lq = 2048).
Attention is permutation-invariant over the key axis, so each core builds
its key/value sequence as [my-half | other-half]: the my-half slice of x
serves as q source, residual, and the first half of k/v — no redundant DMA
and no cross-core collectives.

Per-core program:
  1. x is DMA'd through staging tiles; bn_stats accumulates GroupNorm stats
     on DVE while GpSimd copies x into fp32r tiles (walrus requires every
     fp32r-matmul operand to be produced by an fp32r-rounding instruction).
     All weights arrive in one packed DMA, small vectors in another.
  2. Group stats: selector-matmul reduces per-partition stats over the
     partition dim, a second selector-matmul broadcasts group stats back.
     The norm affine h = A*x + B is folded into the q/k/v weights:
     W_eff = W * A[ci], b_eff = W @ Bn + b.  q additionally absorbs the
     1/sqrt(C) score scale.  v's bias is deferred to the output bias
     (softmax rows sum to 1 and proj is linear, so it becomes wp@bv).
  3. q[c,i], k[c,j] and vT[j,c] via fp32r matmuls from (rounded) raw x.
  4. Attention over j-blocks of 128: scores sT[j,i] = k.q (PSUM), exp on
     ACT -> SBUF (fp32r), AV matmul accumulates unnormalized h2[c,i]; the
     softmax denominator accumulates as eacc += e alternating between the
     Vector and GpSimd engines (two accumulators), reduced+broadcast by an
     all-ones matmul per i-chunk.  No max-subtraction (scores are bounded,
     fp32 exp cannot overflow here).
  5. Normalization is applied AFTER the proj matmul (scaling by 1/denom[i]
     commutes with the channel-mixing matmul), so the slow DVE reciprocal
     never blocks the PE: out = x + proj(h2)/denom + (wp@bv + pb).
     Each i-chunk's epilogue is emitted after the next chunk's j-loop has
     started so the PE never drains at chunk boundaries.
"""

import numpy as np

B, C, L = 4, 256, 4096
NCORES = 8
LQ = L // 2
EPS = 1e-6

_CACHE = {}


def _build_program():
    import concourse.bacc as bacc
    import concourse.tile as tile
    from concourse import mybir
    from contextlib import ExitStack

    f32 = mybir.dt.float32
    f32r = mybir.dt.float32r
    AF = mybir.ActivationFunctionType

    nc = bacc.Bacc()

    # DRAM parameters
    xq_d = nc.dram_tensor("xq", [2, 128, LQ], f32, kind="ExternalInput")
    xo_d = nc.dram_tensor("xo", [2, 128, LQ], f32, kind="ExternalInput")
    wall_d = nc.dram_tensor("wall", [4, 2, 128, C], f32, kind="ExternalInput")
    sm_d = nc.dram_tensor("sm", [128, 28], f32, kind="ExternalInput")
    selb_d = nc.dram_tensor("selb", [16, 128], f32, kind="ExternalInput")
    y_d = nc.dram_tensor("y", [2, 128, LQ], f32, kind="ExternalOutput")

    with tile.TileContext(nc) as tc, ExitStack() as ctx:
        const = ctx.enter_context(tc.tile_pool(name="const", bufs=1))
        stage = ctx.enter_context(tc.tile_pool(name="stage", bufs=4))
        work = ctx.enter_context(tc.tile_pool(name="work", bufs=2))
        epool = ctx.enter_context(tc.tile_pool(name="epool", bufs=6))
        ps = ctx.enter_context(tc.tile_pool(name="ps", bufs=4, space="PSUM"))
        acc = ctx.enter_context(tc.tile_pool(name="acc", bufs=2, space="PSUM"))

        # ---- persistent SBUF tiles (fp32r ones feed the tensor engine) ----
        # xkv_r columns 0:LQ are this core's query half (q source + residual).
        xkv_r = const.tile([128, 2, L], f32r, name="xkv_r")
        q_sb = const.tile([128, 2, LQ], f32r, name="q_sb")
        k_sb = const.tile([128, 2, L], f32r, name="k_sb")
        vT_sb = const.tile([128, 32, C], f32r, name="vT_sb")
        wall_sb = const.tile([128, 4, 2, C], f32, name="wall_sb")
        wq_e = const.tile([128, 2, C], f32r, name="wq_e")
        wk_e = const.tile([128, 2, C], f32r, name="wk_e")
        wv_e = const.tile([128, 2, C], f32r, name="wv_e")
        wp_e = const.tile([128, 2, C], f32r, name="wp_e")
        sm_sb = const.tile([128, 28], f32, name="sm_sb")
        selb_sb = const.tile([16, 128], f32, name="selb_sb")
        smm = const.tile([128, 4], f32, name="smm")
        pstat = const.tile([128, 4], f32, name="pstat")
        gsb = const.tile([16, 4], f32, name="gsb")
        A_sb = const.tile([128, 2], f32, name="A_sb")
        Bn_sb = const.tile([128, 2], f32, name="Bn_sb")
        bq_e = const.tile([128, 2], f32, name="bq_e")
        bk_e = const.tile([128, 2], f32, name="bk_e")
        bv_e = const.tile([128, 2], f32, name="bv_e")
        pbc_sb = const.tile([128, 2], f32, name="pbc_sb")
        eps_sb = const.tile([128, 1], f32, name="eps_sb")
        onesmat = const.tile([128, 128], f32r, name="onesmat")
        onesmf = const.tile([128, 128], f32, name="onesmf")
        stats0 = const.tile([128, 8, 6], f32, name="stats0")
        stats1 = const.tile([128, 8, 6], f32, name="stats1")
        statst = (stats0, stats1)

        # packed views
        wqT_sb = wall_sb[:, 0]
        wkT_sb = wall_sb[:, 1]
        wvT_sb = wall_sb[:, 2]
        wpT_sb = wall_sb[:, 3]
        g2_sb = sm_sb[:, 0:2]
        b2_sb = sm_sb[:, 2:4]
        qb2_sb = sm_sb[:, 4:6]
        kb2_sb = sm_sb[:, 6:8]
        vb2_sb = sm_sb[:, 8:10]
        pb2_sb = sm_sb[:, 10:12]
        selg_sb = sm_sb[:, 12:28]

        nc.vector.memset(eps_sb, EPS)
        nc.vector.memset(onesmf, 1.0)
        nc.vector.tensor_copy(out=onesmat, in_=onesmf)

        # prime the ACT function tables (Sqrt/Square/Exp) during the DMA wait
        # so no table-load lands on the post-stats critical chain
        tprime = work.tile([128, 1], f32, name="tprime", tag="tprime")
        nc.scalar.activation(out=tprime, in_=eps_sb, func=AF.Exp)
        nc.scalar.activation(out=tprime, in_=eps_sb, func=AF.Sqrt)

        # ---- x DMAs first (stats critical path); bn_stats on DVE, ----
        # ---- fp32r rounding copies on the otherwise-idle GpSimd    ----
        # chunk layout: s=0 -> my half, s=1 -> other half (1MB transfers)
        for s in range(2):
            src_d = xq_d if s == 0 else xo_d
            for cb in range(2):
                stg = stage.tile([128, LQ], f32, name="stg", tag="stg", bufs=4)
                nc.sync.dma_start(out=stg, in_=src_d[cb][:, :])
                for u in range(4):
                    nc.vector.bn_stats(
                        out=statst[cb][:, 4 * s + u, :],
                        in_=stg[:, u * 512 : (u + 1) * 512],
                    )
                nc.gpsimd.tensor_copy(
                    out=xkv_r[:, cb, s * LQ : (s + 1) * LQ], in_=stg
                )

        # packed weight + small-vector DMAs (HWDGE queue, right after x)
        nc.sync.dma_start(
            out=wall_sb, in_=wall_d.rearrange("w c p n -> p w c n")
        )
        nc.sync.dma_start(out=sm_sb, in_=sm_d[:, :])
        nc.sync.dma_start(out=selb_sb, in_=selb_d[:, :])

        # smm layout: cols [mean_b0, mean_b1, Ex2_b0, Ex2_b1]
        for cb in range(2):
            mv = work.tile([128, 2], f32, name="mv", tag="mv")
            nc.vector.bn_aggr(out=mv, in_=statst[cb])
            tmp1 = work.tile([128, 1], f32, name="tmp1", tag="tmp1")
            nc.vector.tensor_copy(out=smm[:, cb : cb + 1], in_=mv[:, 0:1])
            nc.vector.tensor_mul(out=tmp1, in0=mv[:, 0:1], in1=mv[:, 0:1])
            nc.vector.tensor_add(
                out=smm[:, 2 + cb : 3 + cb], in0=mv[:, 1:2], in1=tmp1
            )

        # group-reduce over partitions (selector matmul), then broadcast back
        gps = ps.tile([16, 4], f32, name="gps", tag="ps")
        nc.tensor.matmul(out=gps, lhsT=selg_sb, rhs=smm, start=True, stop=True)
        nc.vector.tensor_copy(out=gsb, in_=gps)
        pst = ps.tile([128, 4], f32, name="pst", tag="ps")
        nc.tensor.matmul(out=pst, lhsT=selb_sb, rhs=gsb, start=True, stop=True)
        nc.vector.tensor_copy(out=pstat, in_=pst)

        # per-channel affine A (scale) and Bn (shift), both c-blocks at once
        mean2 = pstat[:, 0:2]
        ex2 = pstat[:, 2:4]
        mm2 = work.tile([128, 2], f32, name="mm2", tag="mm2")
        nc.vector.tensor_mul(out=mm2, in0=mean2, in1=mean2)
        var2 = work.tile([128, 2], f32, name="var2", tag="var2")
        nc.vector.tensor_sub(out=var2, in0=ex2, in1=mm2)
        std2 = work.tile([128, 2], f32, name="std2", tag="std2")
        nc.scalar.activation(
            out=std2, in_=var2, func=AF.Sqrt, bias=eps_sb[:, 0:1], scale=1.0
        )
        rstd2 = work.tile([128, 2], f32, name="rstd2", tag="rstd2")
        nc.vector.reciprocal(out=rstd2, in_=std2)
        nc.vector.tensor_mul(out=A_sb, in0=rstd2, in1=g2_sb)
        tB = work.tile([128, 2], f32, name="tB", tag="tB")
        nc.vector.tensor_mul(out=tB, in0=mean2, in1=A_sb)
        nc.vector.tensor_sub(out=Bn_sb, in0=b2_sb, in1=tB)

        # effective weights (norm scale folded in; q also takes C**-0.5)
        for cb in range(2):
            nc.vector.tensor_scalar(
                out=wq_e[:, cb, :],
                in0=wqT_sb[:, cb, :],
                scalar1=A_sb[:, cb : cb + 1],
                scalar2=C**-0.5,
                op0=mybir.AluOpType.mult,
                op1=mybir.AluOpType.mult,
            )
            nc.vector.tensor_scalar_mul(
                out=wk_e[:, cb, :], in0=wkT_sb[:, cb, :], scalar1=A_sb[:, cb : cb + 1]
            )
            nc.vector.tensor_scalar_mul(
                out=wv_e[:, cb, :], in0=wvT_sb[:, cb, :], scalar1=A_sb[:, cb : cb + 1]
            )
            nc.gpsimd.tensor_copy(out=wp_e[:, cb, :], in_=wpT_sb[:, cb, :])

        # effective biases: b_eff = W @ Bn + b   (q: then * C**-0.5)
        for wraw, braw, beff, qscale in (
            (wqT_sb, qb2_sb, bq_e, True),
            (wkT_sb, kb2_sb, bk_e, False),
            (wvT_sb, vb2_sb, bv_e, False),
        ):
            for ob in range(2):
                bps = acc.tile([128, 1], f32, name="bps", tag="acc")
                for cb in range(2):
                    nc.tensor.matmul(
                        out=bps,
                        lhsT=wraw[:, cb, ob * 128 : (ob + 1) * 128],
                        rhs=Bn_sb[:, cb : cb + 1],
                        start=(cb == 0),
                        stop=(cb == 1),
                    )
                if qscale:
                    nc.vector.tensor_scalar(
                        out=beff[:, ob : ob + 1],
                        in0=bps,
                        scalar1=braw[:, ob : ob + 1],
                        scalar2=C**-0.5,
                        op0=mybir.AluOpType.add,
                        op1=mybir.AluOpType.mult,
                    )
                else:
                    nc.vector.tensor_scalar_add(
                        out=beff[:, ob : ob + 1], in0=bps, scalar1=braw[:, ob : ob + 1]
                    )

        # output bias: pbc = wp @ bv_e + pb  (v's bias commutes through softmax)
        for ob in range(2):
            pvb = acc.tile([128, 1], f32, name="pvb", tag="acc")
            for cb in range(2):
                nc.tensor.matmul(
                    out=pvb,
                    lhsT=wpT_sb[:, cb, ob * 128 : (ob + 1) * 128],
                    rhs=bv_e[:, cb : cb + 1],
                    start=(cb == 0),
                    stop=(cb == 1),
                )
            nc.vector.tensor_scalar_add(
                out=pbc_sb[:, ob : ob + 1], in0=pvb, scalar1=pb2_sb[:, ob : ob + 1]
            )

        # ---- k, vT, then q projections (from raw x, eff weights) ----
        for ob in range(2):
            for chi in range(L // 512):
                pq = ps.tile([128, 512], f32, name="pq", tag="ps")
                for cb in range(2):
                    nc.tensor.matmul(
                        out=pq,
                        lhsT=wk_e[:, cb, ob * 128 : (ob + 1) * 128],
                        rhs=xkv_r[:, cb, chi * 512 : (chi + 1) * 512],
                        start=(cb == 0),
                        stop=(cb == 1),
                    )
                nc.vector.tensor_scalar_add(
                    out=k_sb[:, ob, chi * 512 : (chi + 1) * 512],
                    in0=pq,
                    scalar1=bk_e[:, ob : ob + 1],
                )

        for jb in range(32):
            pv = ps.tile([128, C], f32, name="pv", tag="ps")
            for cb in range(2):
                nc.tensor.matmul(
                    out=pv,
                    lhsT=xkv_r[:, cb, jb * 128 : (jb + 1) * 128],
                    rhs=wv_e[:, cb, :],
                    start=(cb == 0),
                    stop=(cb == 1),
                )
            nc.vector.tensor_copy(out=vT_sb[:, jb, :], in_=pv)

        for ob in range(2):
            for chi in range(LQ // 512):
                pq = ps.tile([128, 512], f32, name="pq", tag="ps")
                for cb in range(2):
                    nc.tensor.matmul(
                        out=pq,
                        lhsT=wq_e[:, cb, ob * 128 : (ob + 1) * 128],
                        rhs=xkv_r[:, cb, chi * 512 : (chi + 1) * 512],
                        start=(cb == 0),
                        stop=(cb == 1),
                    )
                nc.vector.tensor_scalar_add(
                    out=q_sb[:, ob, chi * 512 : (chi + 1) * 512],
                    in0=pq,
                    scalar1=bq_e[:, ob : ob + 1],
                )

        # ---- attention; the epilogue of chunk ic is emitted in stages ----
        # ---- interleaved into chunk ic+1's early j-loop so neither   ----
        # ---- the PE nor the in-order DVE accumulation chain stalls   ----
        def make_epilogue(ic, h2, eacc_d, eacc_g):
            st = {}

            def stage_a():  # h2r copies (gate proj MMs) + denominator MMs
                h2r = work.tile([128, 2, 512], f32r, name="h2r", tag="h2r")
                for cb in range(2):
                    nc.vector.tensor_copy(out=h2r[:, cb, :], in_=h2[:, cb, :])
                bc = ps.tile([128, 512], f32, name="bc", tag="ps")
                nc.tensor.matmul(
                    out=bc, lhsT=onesmat, rhs=eacc_d, start=True, stop=False
                )
                nc.tensor.matmul(
                    out=bc, lhsT=onesmat, rhs=eacc_g, start=False, stop=True
                )
                st["h2r"], st["bc"] = h2r, bc

            def stage_b():  # slow reciprocal, off the PE path
                rcp = work.tile([128, 512], f32, name="rcp", tag="rcp", bufs=1)
                nc.vector.reciprocal(out=rcp, in_=st["bc"])
                st["rcp"] = rcp

            def po_osb(ob):
                po = ps.tile([128, 512], f32, name="po", tag="ps")
                for cb in range(2):
                    nc.tensor.matmul(
                        out=po,
                        lhsT=wp_e[:, cb, ob * 128 : (ob + 1) * 128],
                        rhs=st["h2r"][:, cb, :],
                        start=(cb == 0),
                        stop=(cb == 1),
                    )
                osb = work.tile([128, 512], f32, name="osb", tag="osb")
                nc.vector.tensor_mul(out=osb, in0=po, in1=st["rcp"])
                nc.vector.tensor_scalar_add(
                    out=osb, in0=osb, scalar1=pbc_sb[:, ob : ob + 1]
                )
                nc.vector.tensor_add(
                    out=osb,
                    in0=osb,
                    in1=xkv_r[:, ob, ic * 512 : (ic + 1) * 512].bitcast(f32),
                )
                nc.sync.dma_start(
                    out=y_d[ob][:, ic * 512 : (ic + 1) * 512], in_=osb
                )

            return [stage_a, stage_b, lambda: po_osb(0), lambda: po_osb(1)]

        stage_at = {0: 0, 1: 1, 3: 2, 4: 3}
        pending = None
        for ic in range(LQ // 512):
            h2 = acc.tile([128, 2, 512], f32, name="h2", tag="acc")
            eacc_d = work.tile([128, 512], f32r, name="eacc_d", tag="eacc_d")
            eacc_g = work.tile([128, 512], f32r, name="eacc_g", tag="eacc_g")
            for jb in range(32):
                sc_t = ps.tile([128, 512], f32, name="sc_t", tag="ps")
                for cb in range(2):
                    nc.tensor.matmul(
                        out=sc_t,
                        lhsT=k_sb[:, cb, jb * 128 : (jb + 1) * 128],
                        rhs=q_sb[:, cb, ic * 512 : (ic + 1) * 512],
                        start=(cb == 0),
                        stop=(cb == 1),
                    )
                e_t = epool.tile([128, 512], f32r, name="e_t", tag="e")
                nc.scalar.activation(out=e_t, in_=sc_t, func=AF.Exp)
                for cb in range(2):
                    nc.tensor.matmul(
                        out=h2[:, cb, :],
                        lhsT=vT_sb[:, jb, cb * 128 : (cb + 1) * 128],
                        rhs=e_t,
                        start=(jb == 0),
                        stop=(jb == 31),
                    )
                if pending is not None and jb in stage_at:
                    pending[stage_at[jb]]()
                eng = nc.vector if jb % 2 == 0 else nc.gpsimd
                eacc = eacc_d if jb % 2 == 0 else eacc_g
                if jb < 2:
                    eng.tensor_copy(out=eacc, in_=e_t.bitcast(f32))
                else:
                    eng.tensor_add(
                        out=eacc,
                        in0=eacc.bitcast(f32),
                        in1=e_t.bitcast(f32),
                    )
            pending = make_epilogue(ic, h2, eacc_d, eacc_g)
        for stage in pending:
            stage()

    nc.compile()
    return nc


def _host_inputs(x, norm_g, norm_b, q_w, q_b, k_w, k_b, v_w, v_b, proj_w, proj_b):
    """Build the per-core input maps."""
    f = np.float32
    x = np.ascontiguousarray(np.asarray(x, dtype=f))

    def pack2(v):  # [256] -> [128, 2]  (col cb = v[cb*128 + p])
        return np.asarray(v, dtype=f).reshape(2, 128).T

    def packw(w):  # [Cout, Cin] -> wT [2, 128, C]  (wT[cb, p, o] = w[o, cb*128+p])
        return np.asarray(w, dtype=f).T.reshape(2, 128, C)

    selg = np.zeros((128, 16), dtype=f)
    selg[np.arange(128), np.arange(128) // 8] = 0.125
    selb = np.zeros((16, 128), dtype=f)
    selb[np.arange(128) // 8, np.arange(128)] = 1.0

    wall = np.ascontiguousarray(
        np.stack([packw(q_w), packw(k_w), packw(v_w), packw(proj_w)])
    )
    sm = np.ascontiguousarray(
        np.concatenate(
            [
                pack2(norm_g),
                pack2(norm_b),
                pack2(q_b),
                pack2(k_b),
                pack2(v_b),
                pack2(proj_b),
                selg,
            ],
            axis=1,
        )
    )
    shared = {"wall": wall, "sm": sm, "selb": selb}
    in_maps = []
    for core in range(NCORES):
        b, h = core // 2, core % 2
        m = dict(shared)
        m["xq"] = np.ascontiguousarray(
            x[b][:, h * LQ : (h + 1) * LQ].reshape(2, 128, LQ)
        )
        m["xo"] = np.ascontiguousarray(
            x[b][:, (1 - h) * LQ : (2 - h) * LQ].reshape(2, 128, LQ)
        )
        in_maps.append(m)
    return in_maps


def kernel(**inputs) -> np.ndarray:
    from concourse.bass_utils import run_bass_kernel_spmd

    if "nc" not in _CACHE:
        _CACHE["nc"] = _build_program()
    nc = _CACHE["nc"]

    in_maps = _host_inputs(**inputs)
    res = run_bass_kernel_spmd(nc, in_maps, list(range(NCORES)))

    out = np.empty((B, C, L), dtype=np.float32)
    for core in range(NCORES):
        b, h = core // 2, core % 2
        out[b][:, h * LQ : (h + 1) * LQ] = res.results[core]["y"].reshape(C, LQ)
    return out


# revision 37
# speedup vs baseline: 1.0302x; 1.0302x over previous
"""AttnBlock (GroupNorm + single-head global attention + residual) on 8 trn2 cores.

Sharding: core c handles batch b = c//2, query-half h = c%2 (Lq = 2048).
Attention is permutation-invariant over the key axis, so each core builds
its key/value sequence as [my-half | other-half]: the my-half slice of x
serves as q source, residual, and the first half of k/v — no redundant DMA
and no cross-core collectives.

Per-core program:
  1. x is DMA'd through staging tiles; bn_stats accumulates GroupNorm stats
     on DVE while GpSimd copies x into fp32r tiles (walrus requires every
     fp32r-matmul operand to be produced by an fp32r-rounding instruction).
     All weights arrive in one packed DMA, small vectors in another.
  2. Group stats: selector-matmul reduces per-partition stats over the
     partition dim, a second selector-matmul broadcasts group stats back.
     The norm affine h = A*x + B is folded into the q/k/v weights:
     W_eff = W * A[ci], b_eff = W @ Bn + b.  q additionally absorbs the
     1/sqrt(C) score scale.  v's bias is deferred to the output bias
     (softmax rows sum to 1 and proj is linear, so it becomes wp@bv).
  3. q[c,i], k[c,j] and vT[j,c] via fp32r matmuls from (rounded) raw x.
  4. Attention over j-blocks of 128: scores sT[j,i] = k.q (PSUM), exp on
     ACT -> SBUF (fp32r), AV matmul accumulates unnormalized h2[c,i]; the
     softmax denominator accumulates as eacc += e alternating between the
     Vector and GpSimd engines (two accumulators), reduced+broadcast by an
     all-ones matmul per i-chunk.  No max-subtraction (scores are bounded,
     fp32 exp cannot overflow here).
  5. Normalization is applied AFTER the proj matmul (scaling by 1/denom[i]
     commutes with the channel-mixing matmul), so the slow DVE reciprocal
     never blocks the PE: out = x + proj(h2)/denom + (wp@bv + pb).
     Each i-chunk's epilogue is emitted after the next chunk's j-loop has
     started so the PE never drains at chunk boundaries.
"""

import numpy as np

B, C, L = 4, 256, 4096
NCORES = 8
LQ = L // 2
EPS = 1e-6

_CACHE = {}


def _build_program():
    import concourse.bacc as bacc
    import concourse.tile as tile
    from concourse import mybir
    from contextlib import ExitStack

    f32 = mybir.dt.float32
    f32r = mybir.dt.float32r
    bf16 = mybir.dt.bfloat16
    AF = mybir.ActivationFunctionType

    nc = bacc.Bacc()

    # DRAM parameters
    xq_d = nc.dram_tensor("xq", [2, 128, LQ], f32, kind="ExternalInput")
    xo_d = nc.dram_tensor("xo", [2, 128, LQ], f32, kind="ExternalInput")
    wall_d = nc.dram_tensor("wall", [4, 2, 128, C], f32, kind="ExternalInput")
    sm_d = nc.dram_tensor("sm", [128, 28], f32, kind="ExternalInput")
    selb_d = nc.dram_tensor("selb", [16, 128], f32, kind="ExternalInput")
    y_d = nc.dram_tensor("y", [2, 128, LQ], f32, kind="ExternalOutput")

    with tile.TileContext(nc) as tc, ExitStack() as ctx:
        const = ctx.enter_context(tc.tile_pool(name="const", bufs=1))
        stage = ctx.enter_context(tc.tile_pool(name="stage", bufs=4))
        work = ctx.enter_context(tc.tile_pool(name="work", bufs=2))
        epool = ctx.enter_context(tc.tile_pool(name="epool", bufs=6))
        ps = ctx.enter_context(tc.tile_pool(name="ps", bufs=4, space="PSUM"))
        acc = ctx.enter_context(tc.tile_pool(name="acc", bufs=2, space="PSUM"))

        # ---- persistent SBUF tiles (fp32r ones feed the tensor engine) ----
        # xkv_r columns 0:LQ are this core's query half (q source + residual).
        xkv_r = const.tile([128, 2, L], f32r, name="xkv_r")
        q_sb = const.tile([128, 2, LQ], f32r, name="q_sb")
        k_sb = const.tile([128, 2, L], f32r, name="k_sb")
        vT_sb = const.tile([128, 32, C], bf16, name="vT_sb")
        wall_sb = const.tile([128, 4, 2, C], f32, name="wall_sb")
        wq_e = const.tile([128, 2, C], f32r, name="wq_e")
        wk_e = const.tile([128, 2, C], f32r, name="wk_e")
        wv_e = const.tile([128, 2, C], f32r, name="wv_e")
        wp_e = const.tile([128, 2, C], f32r, name="wp_e")
        sm_sb = const.tile([128, 28], f32, name="sm_sb")
        selb_sb = const.tile([16, 128], f32, name="selb_sb")
        smm = const.tile([128, 4], f32, name="smm")
        pstat = const.tile([128, 4], f32, name="pstat")
        gsb = const.tile([16, 4], f32, name="gsb")
        A_sb = const.tile([128, 2], f32, name="A_sb")
        Bn_sb = const.tile([128, 2], f32, name="Bn_sb")
        bq_e = const.tile([128, 2], f32, name="bq_e")
        bk_e = const.tile([128, 2], f32, name="bk_e")
        bv_e = const.tile([128, 2], f32, name="bv_e")
        pbc_sb = const.tile([128, 2], f32, name="pbc_sb")
        eps_sb = const.tile([128, 1], f32, name="eps_sb")
        onesmat = const.tile([128, 128], f32r, name="onesmat")
        onesmf = const.tile([128, 128], f32, name="onesmf")
        stats0 = const.tile([128, 8, 6], f32, name="stats0")
        stats1 = const.tile([128, 8, 6], f32, name="stats1")
        statst = (stats0, stats1)

        # packed views
        wqT_sb = wall_sb[:, 0]
        wkT_sb = wall_sb[:, 1]
        wvT_sb = wall_sb[:, 2]
        wpT_sb = wall_sb[:, 3]
        g2_sb = sm_sb[:, 0:2]
        b2_sb = sm_sb[:, 2:4]
        qb2_sb = sm_sb[:, 4:6]
        kb2_sb = sm_sb[:, 6:8]
        vb2_sb = sm_sb[:, 8:10]
        pb2_sb = sm_sb[:, 10:12]
        selg_sb = sm_sb[:, 12:28]

        nc.vector.memset(eps_sb, EPS)
        nc.vector.memset(onesmf, 1.0)
        nc.vector.tensor_copy(out=onesmat, in_=onesmf)

        # prime the ACT function tables (Sqrt/Square/Exp) during the DMA wait
        # so no table-load lands on the post-stats critical chain
        tprime = work.tile([128, 1], f32, name="tprime", tag="tprime")
        nc.scalar.activation(out=tprime, in_=eps_sb, func=AF.Exp)
        nc.scalar.activation(out=tprime, in_=eps_sb, func=AF.Sqrt)

        # ---- x DMAs first (stats critical path); bn_stats on DVE, ----
        # ---- fp32r rounding copies on the otherwise-idle GpSimd    ----
        # chunk layout: s=0 -> my half, s=1 -> other half (1MB transfers)
        for s in range(2):
            src_d = xq_d if s == 0 else xo_d
            for cb in range(2):
                stg = stage.tile([128, LQ], f32, name="stg", tag="stg", bufs=4)
                nc.sync.dma_start(out=stg, in_=src_d[cb][:, :])
                for u in range(4):
                    nc.vector.bn_stats(
                        out=statst[cb][:, 4 * s + u, :],
                        in_=stg[:, u * 512 : (u + 1) * 512],
                    )
                nc.gpsimd.tensor_copy(
                    out=xkv_r[:, cb, s * LQ : (s + 1) * LQ], in_=stg
                )

        # packed weight + small-vector DMAs (HWDGE queue, right after x)
        nc.sync.dma_start(
            out=wall_sb, in_=wall_d.rearrange("w c p n -> p w c n")
        )
        nc.sync.dma_start(out=sm_sb, in_=sm_d[:, :])
        nc.sync.dma_start(out=selb_sb, in_=selb_d[:, :])

        # smm layout: cols [mean_b0, mean_b1, Ex2_b0, Ex2_b1]
        for cb in range(2):
            mv = work.tile([128, 2], f32, name="mv", tag="mv")
            nc.vector.bn_aggr(out=mv, in_=statst[cb])
            tmp1 = work.tile([128, 1], f32, name="tmp1", tag="tmp1")
            nc.vector.tensor_copy(out=smm[:, cb : cb + 1], in_=mv[:, 0:1])
            nc.vector.tensor_mul(out=tmp1, in0=mv[:, 0:1], in1=mv[:, 0:1])
            nc.vector.tensor_add(
                out=smm[:, 2 + cb : 3 + cb], in0=mv[:, 1:2], in1=tmp1
            )

        # group-reduce over partitions (selector matmul), then broadcast back
        gps = ps.tile([16, 4], f32, name="gps", tag="ps")
        nc.tensor.matmul(out=gps, lhsT=selg_sb, rhs=smm, start=True, stop=True)
        nc.vector.tensor_copy(out=gsb, in_=gps)
        pst = ps.tile([128, 4], f32, name="pst", tag="ps")
        nc.tensor.matmul(out=pst, lhsT=selb_sb, rhs=gsb, start=True, stop=True)
        nc.vector.tensor_copy(out=pstat, in_=pst)

        # per-channel affine A (scale) and Bn (shift), both c-blocks at once
        mean2 = pstat[:, 0:2]
        ex2 = pstat[:, 2:4]
        mm2 = work.tile([128, 2], f32, name="mm2", tag="mm2")
        nc.vector.tensor_mul(out=mm2, in0=mean2, in1=mean2)
        var2 = work.tile([128, 2], f32, name="var2", tag="var2")
        nc.vector.tensor_sub(out=var2, in0=ex2, in1=mm2)
        std2 = work.tile([128, 2], f32, name="std2", tag="std2")
        nc.scalar.activation(
            out=std2, in_=var2, func=AF.Sqrt, bias=eps_sb[:, 0:1], scale=1.0
        )
        rstd2 = work.tile([128, 2], f32, name="rstd2", tag="rstd2")
        nc.vector.reciprocal(out=rstd2, in_=std2)
        nc.vector.tensor_mul(out=A_sb, in0=rstd2, in1=g2_sb)
        tB = work.tile([128, 2], f32, name="tB", tag="tB")
        nc.vector.tensor_mul(out=tB, in0=mean2, in1=A_sb)
        nc.vector.tensor_sub(out=Bn_sb, in0=b2_sb, in1=tB)

        # effective weights (norm scale folded in; q also takes C**-0.5)
        for cb in range(2):
            nc.vector.tensor_scalar(
                out=wq_e[:, cb, :],
                in0=wqT_sb[:, cb, :],
                scalar1=A_sb[:, cb : cb + 1],
                scalar2=C**-0.5,
                op0=mybir.AluOpType.mult,
                op1=mybir.AluOpType.mult,
            )
            nc.vector.tensor_scalar_mul(
                out=wk_e[:, cb, :], in0=wkT_sb[:, cb, :], scalar1=A_sb[:, cb : cb + 1]
            )
            nc.vector.tensor_scalar_mul(
                out=wv_e[:, cb, :], in0=wvT_sb[:, cb, :], scalar1=A_sb[:, cb : cb + 1]
            )
            nc.gpsimd.tensor_copy(out=wp_e[:, cb, :], in_=wpT_sb[:, cb, :])

        # effective biases: b_eff = W @ Bn + b   (q: then * C**-0.5)
        for wraw, braw, beff, qscale in (
            (wqT_sb, qb2_sb, bq_e, True),
            (wkT_sb, kb2_sb, bk_e, False),
            (wvT_sb, vb2_sb, bv_e, False),
        ):
            for ob in range(2):
                bps = acc.tile([128, 1], f32, name="bps", tag="acc")
                for cb in range(2):
                    nc.tensor.matmul(
                        out=bps,
                        lhsT=wraw[:, cb, ob * 128 : (ob + 1) * 128],
                        rhs=Bn_sb[:, cb : cb + 1],
                        start=(cb == 0),
                        stop=(cb == 1),
                    )
                if qscale:
                    nc.vector.tensor_scalar(
                        out=beff[:, ob : ob + 1],
                        in0=bps,
                        scalar1=braw[:, ob : ob + 1],
                        scalar2=C**-0.5,
                        op0=mybir.AluOpType.add,
                        op1=mybir.AluOpType.mult,
                    )
                else:
                    nc.vector.tensor_scalar_add(
                        out=beff[:, ob : ob + 1], in0=bps, scalar1=braw[:, ob : ob + 1]
                    )

        # output bias: pbc = wp @ bv_e + pb  (v's bias commutes through softmax)
        for ob in range(2):
            pvb = acc.tile([128, 1], f32, name="pvb", tag="acc")
            for cb in range(2):
                nc.tensor.matmul(
                    out=pvb,
                    lhsT=wpT_sb[:, cb, ob * 128 : (ob + 1) * 128],
                    rhs=bv_e[:, cb : cb + 1],
                    start=(cb == 0),
                    stop=(cb == 1),
                )
            nc.vector.tensor_scalar_add(
                out=pbc_sb[:, ob : ob + 1], in0=pvb, scalar1=pb2_sb[:, ob : ob + 1]
            )

        # ---- k, vT, then q projections (from raw x, eff weights) ----
        for ob in range(2):
            for chi in range(L // 512):
                pq = ps.tile([128, 512], f32, name="pq", tag="ps")
                for cb in range(2):
                    nc.tensor.matmul(
                        out=pq,
                        lhsT=wk_e[:, cb, ob * 128 : (ob + 1) * 128],
                        rhs=xkv_r[:, cb, chi * 512 : (chi + 1) * 512],
                        start=(cb == 0),
                        stop=(cb == 1),
                    )
                nc.vector.tensor_scalar_add(
                    out=k_sb[:, ob, chi * 512 : (chi + 1) * 512],
                    in0=pq,
                    scalar1=bk_e[:, ob : ob + 1],
                )

        for jb in range(32):
            pv = ps.tile([128, C], f32, name="pv", tag="ps")
            for cb in range(2):
                nc.tensor.matmul(
                    out=pv,
                    lhsT=xkv_r[:, cb, jb * 128 : (jb + 1) * 128],
                    rhs=wv_e[:, cb, :],
                    start=(cb == 0),
                    stop=(cb == 1),
                )
            nc.vector.tensor_copy(out=vT_sb[:, jb, :], in_=pv)

        for ob in range(2):
            for chi in range(LQ // 512):
                pq = ps.tile([128, 512], f32, name="pq", tag="ps")
                for cb in range(2):
                    nc.tensor.matmul(
                        out=pq,
                        lhsT=wq_e[:, cb, ob * 128 : (ob + 1) * 128],
                        rhs=xkv_r[:, cb, chi * 512 : (chi + 1) * 512],
                        start=(cb == 0),
                        stop=(cb == 1),
                    )
                nc.vector.tensor_scalar_add(
                    out=q_sb[:, ob, chi * 512 : (chi + 1) * 512],
                    in0=pq,
                    scalar1=bq_e[:, ob : ob + 1],
                )

        # ---- attention; the epilogue of chunk ic is emitted in stages ----
        # ---- interleaved into chunk ic+1's early j-loop so neither   ----
        # ---- the PE nor the in-order DVE accumulation chain stalls   ----
        def make_epilogue(ic, h2, eacc_d, eacc_g):
            st = {}

            def stage_a():  # h2r copies (gate proj MMs) + denominator MMs
                h2r = work.tile([128, 2, 512], f32r, name="h2r", tag="h2r")
                for cb in range(2):
                    nc.vector.tensor_copy(out=h2r[:, cb, :], in_=h2[:, cb, :])
                bc = ps.tile([128, 512], f32, name="bc", tag="ps")
                nc.tensor.matmul(
                    out=bc, lhsT=onesmat, rhs=eacc_d, start=True, stop=False
                )
                nc.tensor.matmul(
                    out=bc, lhsT=onesmat, rhs=eacc_g, start=False, stop=True
                )
                st["h2r"], st["bc"] = h2r, bc

            def stage_b():  # slow reciprocal, off the PE path
                rcp = work.tile([128, 512], f32, name="rcp", tag="rcp", bufs=1)
                nc.vector.reciprocal(out=rcp, in_=st["bc"])
                st["rcp"] = rcp

            def po_osb(ob):
                po = ps.tile([128, 512], f32, name="po", tag="ps")
                for cb in range(2):
                    nc.tensor.matmul(
                        out=po,
                        lhsT=wp_e[:, cb, ob * 128 : (ob + 1) * 128],
                        rhs=st["h2r"][:, cb, :],
                        start=(cb == 0),
                        stop=(cb == 1),
                    )
                osb = work.tile([128, 512], f32, name="osb", tag="osb")
                nc.vector.tensor_mul(out=osb, in0=po, in1=st["rcp"])
                nc.vector.tensor_scalar_add(
                    out=osb, in0=osb, scalar1=pbc_sb[:, ob : ob + 1]
                )
                nc.vector.tensor_add(
                    out=osb,
                    in0=osb,
                    in1=xkv_r[:, ob, ic * 512 : (ic + 1) * 512].bitcast(f32),
                )
                nc.sync.dma_start(
                    out=y_d[ob][:, ic * 512 : (ic + 1) * 512], in_=osb
                )

            return [stage_a, stage_b, lambda: po_osb(0), lambda: po_osb(1)]

        stage_at = {0: 0, 1: 1, 3: 2, 4: 3}
        pending = None
        for ic in range(LQ // 512):
            h2 = acc.tile([128, 2, 512], f32, name="h2", tag="acc")
            eacc_d = work.tile([128, 512], f32r, name="eacc_d", tag="eacc_d")
            eacc_g = work.tile([128, 512], f32r, name="eacc_g", tag="eacc_g")
            for jb in range(32):
                sc_t = ps.tile([128, 512], f32, name="sc_t", tag="ps")
                for cb in range(2):
                    nc.tensor.matmul(
                        out=sc_t,
                        lhsT=k_sb[:, cb, jb * 128 : (jb + 1) * 128],
                        rhs=q_sb[:, cb, ic * 512 : (ic + 1) * 512],
                        start=(cb == 0),
                        stop=(cb == 1),
                    )
                e_t = epool.tile([128, 512], bf16, name="e_t", tag="e")
                nc.scalar.activation(out=e_t, in_=sc_t, func=AF.Exp)
                for cb in range(2):
                    nc.tensor.matmul(
                        out=h2[:, cb, :],
                        lhsT=vT_sb[:, jb, cb * 128 : (cb + 1) * 128],
                        rhs=e_t,
                        start=(jb == 0),
                        stop=(jb == 31),
                    )
                if pending is not None and jb in stage_at:
                    pending[stage_at[jb]]()
                eng = nc.vector if jb % 2 == 0 else nc.gpsimd
                eacc = eacc_d if jb % 2 == 0 else eacc_g
                if jb < 2:
                    eng.tensor_copy(out=eacc, in_=e_t)
                else:
                    eng.tensor_add(
                        out=eacc,
                        in0=eacc.bitcast(f32),
                        in1=e_t,
                    )
            pending = make_epilogue(ic, h2, eacc_d, eacc_g)
        for stage in pending:
            stage()

    nc.compile()
    return nc


def _host_inputs(x, norm_g, norm_b, q_w, q_b, k_w, k_b, v_w, v_b, proj_w, proj_b):
    """Build the per-core input maps."""
    f = np.float32
    x = np.ascontiguousarray(np.asarray(x, dtype=f))

    def pack2(v):  # [256] -> [128, 2]  (col cb = v[cb*128 + p])
        return np.asarray(v, dtype=f).reshape(2, 128).T

    def packw(w):  # [Cout, Cin] -> wT [2, 128, C]  (wT[cb, p, o] = w[o, cb*128+p])
        return np.asarray(w, dtype=f).T.reshape(2, 128, C)

    selg = np.zeros((128, 16), dtype=f)
    selg[np.arange(128), np.arange(128) // 8] = 0.125
    selb = np.zeros((16, 128), dtype=f)
    selb[np.arange(128) // 8, np.arange(128)] = 1.0

    wall = np.ascontiguousarray(
        np.stack([packw(q_w), packw(k_w), packw(v_w), packw(proj_w)])
    )
    sm = np.ascontiguousarray(
        np.concatenate(
            [
                pack2(norm_g),
                pack2(norm_b),
                pack2(q_b),
                pack2(k_b),
                pack2(v_b),
                pack2(proj_b),
                selg,
            ],
            axis=1,
        )
    )
    shared = {"wall": wall, "sm": sm, "selb": selb}
    in_maps = []
    for core in range(NCORES):
        b, h = core // 2, core % 2
        m = dict(shared)
        m["xq"] = np.ascontiguousarray(
            x[b][:, h * LQ : (h + 1) * LQ].reshape(2, 128, LQ)
        )
        m["xo"] = np.ascontiguousarray(
            x[b][:, (1 - h) * LQ : (2 - h) * LQ].reshape(2, 128, LQ)
        )
        in_maps.append(m)
    return in_maps


def kernel(**inputs) -> np.ndarray:
    from concourse.bass_utils import run_bass_kernel_spmd

    if "nc" not in _CACHE:
        _CACHE["nc"] = _build_program()
    nc = _CACHE["nc"]

    in_maps = _host_inputs(**inputs)
    res = run_bass_kernel_spmd(nc, in_maps, list(range(NCORES)))

    out = np.empty((B, C, L), dtype=np.float32)
    for core in range(NCORES):
        b, h = core // 2, core % 2
        out[b][:, h * LQ : (h + 1) * LQ] = res.results[core]["y"].reshape(C, LQ)
    return out


# revision 38
# speedup vs baseline: 1.0482x; 1.0175x over previous
"""AttnBlock (GroupNorm + single-head global attention + residual) on 8 trn2 cores.

Sharding: core c handles batch b = c//2, query-half h = c%2 (Lq = 2048).
Attention is permutation-invariant over the key axis, so each core builds
its key/value sequence as [my-half | other-half]: the my-half slice of x
serves as q source, residual, and the first half of k/v — no redundant DMA
and no cross-core collectives.

Per-core program:
  1. x is DMA'd through staging tiles; bn_stats accumulates GroupNorm stats
     on DVE while GpSimd copies x into fp32r tiles (walrus requires every
     fp32r-matmul operand to be produced by an fp32r-rounding instruction).
     All weights arrive in one packed DMA, small vectors in another.
  2. Group stats: selector-matmul reduces per-partition stats over the
     partition dim, a second selector-matmul broadcasts group stats back.
     The norm affine h = A*x + B is folded into the q/k/v weights:
     W_eff = W * A[ci], b_eff = W @ Bn + b.  q additionally absorbs the
     1/sqrt(C) score scale.  v's bias is deferred to the output bias
     (softmax rows sum to 1 and proj is linear, so it becomes wp@bv).
  3. q[c,i], k[c,j] and vT[j,c] via fp32r matmuls from (rounded) raw x.
  4. Attention over j-blocks of 128: scores sT[j,i] = k.q (PSUM), exp on
     ACT -> SBUF (fp32r), AV matmul accumulates unnormalized h2[c,i]; the
     softmax denominator accumulates as eacc += e alternating between the
     Vector and GpSimd engines (two accumulators), reduced+broadcast by an
     all-ones matmul per i-chunk.  No max-subtraction (scores are bounded,
     fp32 exp cannot overflow here).
  5. Normalization is applied AFTER the proj matmul (scaling by 1/denom[i]
     commutes with the channel-mixing matmul), so the slow DVE reciprocal
     never blocks the PE: out = x + proj(h2)/denom + (wp@bv + pb).
     Each i-chunk's epilogue is emitted after the next chunk's j-loop has
     started so the PE never drains at chunk boundaries.
"""

import numpy as np

B, C, L = 4, 256, 4096
NCORES = 8
LQ = L // 2
EPS = 1e-6

_CACHE = {}


def _build_program():
    import concourse.bacc as bacc
    import concourse.tile as tile
    from concourse import mybir
    from contextlib import ExitStack

    f32 = mybir.dt.float32
    f32r = mybir.dt.float32r
    bf16 = mybir.dt.bfloat16
    AF = mybir.ActivationFunctionType

    nc = bacc.Bacc()

    # DRAM parameters
    xq_d = nc.dram_tensor("xq", [2, 128, LQ], f32, kind="ExternalInput")
    xo_d = nc.dram_tensor("xo", [2, 128, LQ], f32, kind="ExternalInput")
    wall_d = nc.dram_tensor("wall", [4, 2, 128, C], f32, kind="ExternalInput")
    sm_d = nc.dram_tensor("sm", [128, 28], f32, kind="ExternalInput")
    selb_d = nc.dram_tensor("selb", [16, 128], f32, kind="ExternalInput")
    y_d = nc.dram_tensor("y", [2, 128, LQ], f32, kind="ExternalOutput")

    with tile.TileContext(nc) as tc, ExitStack() as ctx:
        const = ctx.enter_context(tc.tile_pool(name="const", bufs=1))
        stage = ctx.enter_context(tc.tile_pool(name="stage", bufs=4))
        work = ctx.enter_context(tc.tile_pool(name="work", bufs=2))
        epool = ctx.enter_context(tc.tile_pool(name="epool", bufs=6))
        ps = ctx.enter_context(tc.tile_pool(name="ps", bufs=4, space="PSUM"))
        acc = ctx.enter_context(tc.tile_pool(name="acc", bufs=2, space="PSUM"))

        # ---- persistent SBUF tiles (fp32r ones feed the tensor engine) ----
        # xkv_r columns 0:LQ are this core's query half (q source + residual).
        xkv_r = const.tile([128, 2, L], f32r, name="xkv_r")
        q_sb = const.tile([128, 2, LQ], bf16, name="q_sb")
        k_sb = const.tile([128, 2, L], bf16, name="k_sb")
        vT_sb = const.tile([128, 32, C], bf16, name="vT_sb")
        wall_sb = const.tile([128, 4, 2, C], f32, name="wall_sb")
        wq_e = const.tile([128, 2, C], f32r, name="wq_e")
        wk_e = const.tile([128, 2, C], f32r, name="wk_e")
        wv_e = const.tile([128, 2, C], f32r, name="wv_e")
        wp_e = const.tile([128, 2, C], f32r, name="wp_e")
        sm_sb = const.tile([128, 28], f32, name="sm_sb")
        selb_sb = const.tile([16, 128], f32, name="selb_sb")
        smm = const.tile([128, 4], f32, name="smm")
        pstat = const.tile([128, 4], f32, name="pstat")
        gsb = const.tile([16, 4], f32, name="gsb")
        A_sb = const.tile([128, 2], f32, name="A_sb")
        Bn_sb = const.tile([128, 2], f32, name="Bn_sb")
        bq_e = const.tile([128, 2], f32, name="bq_e")
        bk_e = const.tile([128, 2], f32, name="bk_e")
        bv_e = const.tile([128, 2], f32, name="bv_e")
        pbc_sb = const.tile([128, 2], f32, name="pbc_sb")
        eps_sb = const.tile([128, 1], f32, name="eps_sb")
        onesmat = const.tile([128, 128], f32r, name="onesmat")
        onesmf = const.tile([128, 128], f32, name="onesmf")
        stats0 = const.tile([128, 8, 6], f32, name="stats0")
        stats1 = const.tile([128, 8, 6], f32, name="stats1")
        statst = (stats0, stats1)

        # packed views
        wqT_sb = wall_sb[:, 0]
        wkT_sb = wall_sb[:, 1]
        wvT_sb = wall_sb[:, 2]
        wpT_sb = wall_sb[:, 3]
        g2_sb = sm_sb[:, 0:2]
        b2_sb = sm_sb[:, 2:4]
        qb2_sb = sm_sb[:, 4:6]
        kb2_sb = sm_sb[:, 6:8]
        vb2_sb = sm_sb[:, 8:10]
        pb2_sb = sm_sb[:, 10:12]
        selg_sb = sm_sb[:, 12:28]

        nc.vector.memset(eps_sb, EPS)
        nc.vector.memset(onesmf, 1.0)
        nc.vector.tensor_copy(out=onesmat, in_=onesmf)

        # prime the ACT function tables (Sqrt/Square/Exp) during the DMA wait
        # so no table-load lands on the post-stats critical chain
        tprime = work.tile([128, 1], f32, name="tprime", tag="tprime")
        nc.scalar.activation(out=tprime, in_=eps_sb, func=AF.Exp)
        nc.scalar.activation(out=tprime, in_=eps_sb, func=AF.Sqrt)

        # ---- x DMAs first (stats critical path); bn_stats on DVE, ----
        # ---- fp32r rounding copies on the otherwise-idle GpSimd    ----
        # chunk layout: s=0 -> my half, s=1 -> other half (1MB transfers)
        for s in range(2):
            src_d = xq_d if s == 0 else xo_d
            for cb in range(2):
                stg = stage.tile([128, LQ], f32, name="stg", tag="stg", bufs=4)
                nc.sync.dma_start(out=stg, in_=src_d[cb][:, :])
                for u in range(4):
                    nc.vector.bn_stats(
                        out=statst[cb][:, 4 * s + u, :],
                        in_=stg[:, u * 512 : (u + 1) * 512],
                    )
                nc.gpsimd.tensor_copy(
                    out=xkv_r[:, cb, s * LQ : (s + 1) * LQ], in_=stg
                )

        # packed weight + small-vector DMAs (HWDGE queue, right after x)
        nc.sync.dma_start(
            out=wall_sb, in_=wall_d.rearrange("w c p n -> p w c n")
        )
        nc.sync.dma_start(out=sm_sb, in_=sm_d[:, :])
        nc.sync.dma_start(out=selb_sb, in_=selb_d[:, :])

        # smm layout: cols [mean_b0, mean_b1, Ex2_b0, Ex2_b1]
        for cb in range(2):
            mv = work.tile([128, 2], f32, name="mv", tag="mv")
            nc.vector.bn_aggr(out=mv, in_=statst[cb])
            tmp1 = work.tile([128, 1], f32, name="tmp1", tag="tmp1")
            nc.vector.tensor_copy(out=smm[:, cb : cb + 1], in_=mv[:, 0:1])
            nc.vector.tensor_mul(out=tmp1, in0=mv[:, 0:1], in1=mv[:, 0:1])
            nc.vector.tensor_add(
                out=smm[:, 2 + cb : 3 + cb], in0=mv[:, 1:2], in1=tmp1
            )

        # group-reduce over partitions (selector matmul), then broadcast back
        gps = ps.tile([16, 4], f32, name="gps", tag="ps")
        nc.tensor.matmul(out=gps, lhsT=selg_sb, rhs=smm, start=True, stop=True)
        nc.vector.tensor_copy(out=gsb, in_=gps)
        pst = ps.tile([128, 4], f32, name="pst", tag="ps")
        nc.tensor.matmul(out=pst, lhsT=selb_sb, rhs=gsb, start=True, stop=True)
        nc.vector.tensor_copy(out=pstat, in_=pst)

        # per-channel affine A (scale) and Bn (shift), both c-blocks at once
        mean2 = pstat[:, 0:2]
        ex2 = pstat[:, 2:4]
        mm2 = work.tile([128, 2], f32, name="mm2", tag="mm2")
        nc.vector.tensor_mul(out=mm2, in0=mean2, in1=mean2)
        var2 = work.tile([128, 2], f32, name="var2", tag="var2")
        nc.vector.tensor_sub(out=var2, in0=ex2, in1=mm2)
        std2 = work.tile([128, 2], f32, name="std2", tag="std2")
        nc.scalar.activation(
            out=std2, in_=var2, func=AF.Sqrt, bias=eps_sb[:, 0:1], scale=1.0
        )
        rstd2 = work.tile([128, 2], f32, name="rstd2", tag="rstd2")
        nc.vector.reciprocal(out=rstd2, in_=std2)
        nc.vector.tensor_mul(out=A_sb, in0=rstd2, in1=g2_sb)
        tB = work.tile([128, 2], f32, name="tB", tag="tB")
        nc.vector.tensor_mul(out=tB, in0=mean2, in1=A_sb)
        nc.vector.tensor_sub(out=Bn_sb, in0=b2_sb, in1=tB)

        # effective weights (norm scale folded in; q also takes C**-0.5)
        for cb in range(2):
            nc.vector.tensor_scalar(
                out=wq_e[:, cb, :],
                in0=wqT_sb[:, cb, :],
                scalar1=A_sb[:, cb : cb + 1],
                scalar2=C**-0.5,
                op0=mybir.AluOpType.mult,
                op1=mybir.AluOpType.mult,
            )
            nc.vector.tensor_scalar_mul(
                out=wk_e[:, cb, :], in0=wkT_sb[:, cb, :], scalar1=A_sb[:, cb : cb + 1]
            )
            nc.vector.tensor_scalar_mul(
                out=wv_e[:, cb, :], in0=wvT_sb[:, cb, :], scalar1=A_sb[:, cb : cb + 1]
            )
            nc.gpsimd.tensor_copy(out=wp_e[:, cb, :], in_=wpT_sb[:, cb, :])

        # effective biases: b_eff = W @ Bn + b   (q: then * C**-0.5)
        for wraw, braw, beff, qscale in (
            (wqT_sb, qb2_sb, bq_e, True),
            (wkT_sb, kb2_sb, bk_e, False),
            (wvT_sb, vb2_sb, bv_e, False),
        ):
            for ob in range(2):
                bps = acc.tile([128, 1], f32, name="bps", tag="acc")
                for cb in range(2):
                    nc.tensor.matmul(
                        out=bps,
                        lhsT=wraw[:, cb, ob * 128 : (ob + 1) * 128],
                        rhs=Bn_sb[:, cb : cb + 1],
                        start=(cb == 0),
                        stop=(cb == 1),
                    )
                if qscale:
                    nc.vector.tensor_scalar(
                        out=beff[:, ob : ob + 1],
                        in0=bps,
                        scalar1=braw[:, ob : ob + 1],
                        scalar2=C**-0.5,
                        op0=mybir.AluOpType.add,
                        op1=mybir.AluOpType.mult,
                    )
                else:
                    nc.vector.tensor_scalar_add(
                        out=beff[:, ob : ob + 1], in0=bps, scalar1=braw[:, ob : ob + 1]
                    )

        # output bias: pbc = wp @ bv_e + pb  (v's bias commutes through softmax)
        for ob in range(2):
            pvb = acc.tile([128, 1], f32, name="pvb", tag="acc")
            for cb in range(2):
                nc.tensor.matmul(
                    out=pvb,
                    lhsT=wpT_sb[:, cb, ob * 128 : (ob + 1) * 128],
                    rhs=bv_e[:, cb : cb + 1],
                    start=(cb == 0),
                    stop=(cb == 1),
                )
            nc.vector.tensor_scalar_add(
                out=pbc_sb[:, ob : ob + 1], in0=pvb, scalar1=pb2_sb[:, ob : ob + 1]
            )

        # ---- k, vT, then q projections (from raw x, eff weights) ----
        for ob in range(2):
            for chi in range(L // 512):
                pq = ps.tile([128, 512], f32, name="pq", tag="ps")
                for cb in range(2):
                    nc.tensor.matmul(
                        out=pq,
                        lhsT=wk_e[:, cb, ob * 128 : (ob + 1) * 128],
                        rhs=xkv_r[:, cb, chi * 512 : (chi + 1) * 512],
                        start=(cb == 0),
                        stop=(cb == 1),
                    )
                nc.scalar.activation(
                    out=k_sb[:, ob, chi * 512 : (chi + 1) * 512],
                    in_=pq,
                    func=AF.Identity,
                    bias=bk_e[:, ob : ob + 1],
                    scale=1.0,
                )

        for jb in range(32):
            pv = ps.tile([128, C], f32, name="pv", tag="ps")
            for cb in range(2):
                nc.tensor.matmul(
                    out=pv,
                    lhsT=xkv_r[:, cb, jb * 128 : (jb + 1) * 128],
                    rhs=wv_e[:, cb, :],
                    start=(cb == 0),
                    stop=(cb == 1),
                )
            nc.scalar.activation(out=vT_sb[:, jb, :], in_=pv, func=AF.Identity)

        for ob in range(2):
            for chi in range(LQ // 512):
                pq = ps.tile([128, 512], f32, name="pq", tag="ps")
                for cb in range(2):
                    nc.tensor.matmul(
                        out=pq,
                        lhsT=wq_e[:, cb, ob * 128 : (ob + 1) * 128],
                        rhs=xkv_r[:, cb, chi * 512 : (chi + 1) * 512],
                        start=(cb == 0),
                        stop=(cb == 1),
                    )
                nc.scalar.activation(
                    out=q_sb[:, ob, chi * 512 : (chi + 1) * 512],
                    in_=pq,
                    func=AF.Identity,
                    bias=bq_e[:, ob : ob + 1],
                    scale=1.0,
                )

        # ---- attention; the epilogue of chunk ic is emitted in stages ----
        # ---- interleaved into chunk ic+1's early j-loop so neither   ----
        # ---- the PE nor the in-order DVE accumulation chain stalls   ----
        def make_epilogue(ic, h2, eacc_d, eacc_g):
            st = {}

            def stage_a():  # h2r copies (gate proj MMs) + denominator MMs
                h2r = work.tile([128, 2, 512], f32r, name="h2r", tag="h2r")
                for cb in range(2):
                    nc.vector.tensor_copy(out=h2r[:, cb, :], in_=h2[:, cb, :])
                bc = ps.tile([128, 512], f32, name="bc", tag="ps")
                nc.tensor.matmul(
                    out=bc, lhsT=onesmat, rhs=eacc_d, start=True, stop=False
                )
                nc.tensor.matmul(
                    out=bc, lhsT=onesmat, rhs=eacc_g, start=False, stop=True
                )
                st["h2r"], st["bc"] = h2r, bc

            def stage_b():  # slow reciprocal, off the PE path
                rcp = work.tile([128, 512], f32, name="rcp", tag="rcp", bufs=1)
                nc.vector.reciprocal(out=rcp, in_=st["bc"])
                st["rcp"] = rcp

            def po_osb(ob):
                po = ps.tile([128, 512], f32, name="po", tag="ps")
                for cb in range(2):
                    nc.tensor.matmul(
                        out=po,
                        lhsT=wp_e[:, cb, ob * 128 : (ob + 1) * 128],
                        rhs=st["h2r"][:, cb, :],
                        start=(cb == 0),
                        stop=(cb == 1),
                    )
                osb = work.tile([128, 512], f32, name="osb", tag="osb")
                nc.vector.tensor_mul(out=osb, in0=po, in1=st["rcp"])
                nc.vector.tensor_scalar_add(
                    out=osb, in0=osb, scalar1=pbc_sb[:, ob : ob + 1]
                )
                nc.vector.tensor_add(
                    out=osb,
                    in0=osb,
                    in1=xkv_r[:, ob, ic * 512 : (ic + 1) * 512].bitcast(f32),
                )
                nc.sync.dma_start(
                    out=y_d[ob][:, ic * 512 : (ic + 1) * 512], in_=osb
                )

            return [stage_a, stage_b, lambda: po_osb(0), lambda: po_osb(1)]

        stage_at = {0: 0, 1: 1, 3: 2, 4: 3}
        pending = None
        for ic in range(LQ // 512):
            h2 = acc.tile([128, 2, 512], f32, name="h2", tag="acc")
            eacc_d = work.tile([128, 512], f32r, name="eacc_d", tag="eacc_d")
            eacc_g = work.tile([128, 512], f32r, name="eacc_g", tag="eacc_g")
            for jb in range(32):
                sc_t = ps.tile([128, 512], f32, name="sc_t", tag="ps")
                for cb in range(2):
                    nc.tensor.matmul(
                        out=sc_t,
                        lhsT=k_sb[:, cb, jb * 128 : (jb + 1) * 128],
                        rhs=q_sb[:, cb, ic * 512 : (ic + 1) * 512],
                        start=(cb == 0),
                        stop=(cb == 1),
                    )
                e_t = epool.tile([128, 512], bf16, name="e_t", tag="e")
                nc.scalar.activation(out=e_t, in_=sc_t, func=AF.Exp)
                for cb in range(2):
                    nc.tensor.matmul(
                        out=h2[:, cb, :],
                        lhsT=vT_sb[:, jb, cb * 128 : (cb + 1) * 128],
                        rhs=e_t,
                        start=(jb == 0),
                        stop=(jb == 31),
                    )
                if pending is not None and jb in stage_at:
                    pending[stage_at[jb]]()
                eng = nc.vector if jb % 2 == 0 else nc.gpsimd
                eacc = eacc_d if jb % 2 == 0 else eacc_g
                if jb < 2:
                    eng.tensor_copy(out=eacc, in_=e_t)
                else:
                    eng.tensor_add(
                        out=eacc,
                        in0=eacc.bitcast(f32),
                        in1=e_t,
                    )
            pending = make_epilogue(ic, h2, eacc_d, eacc_g)
        for stage in pending:
            stage()

    nc.compile()
    return nc


def _host_inputs(x, norm_g, norm_b, q_w, q_b, k_w, k_b, v_w, v_b, proj_w, proj_b):
    """Build the per-core input maps."""
    f = np.float32
    x = np.ascontiguousarray(np.asarray(x, dtype=f))

    def pack2(v):  # [256] -> [128, 2]  (col cb = v[cb*128 + p])
        return np.asarray(v, dtype=f).reshape(2, 128).T

    def packw(w):  # [Cout, Cin] -> wT [2, 128, C]  (wT[cb, p, o] = w[o, cb*128+p])
        return np.asarray(w, dtype=f).T.reshape(2, 128, C)

    selg = np.zeros((128, 16), dtype=f)
    selg[np.arange(128), np.arange(128) // 8] = 0.125
    selb = np.zeros((16, 128), dtype=f)
    selb[np.arange(128) // 8, np.arange(128)] = 1.0

    wall = np.ascontiguousarray(
        np.stack([packw(q_w), packw(k_w), packw(v_w), packw(proj_w)])
    )
    sm = np.ascontiguousarray(
        np.concatenate(
            [
                pack2(norm_g),
                pack2(norm_b),
                pack2(q_b),
                pack2(k_b),
                pack2(v_b),
                pack2(proj_b),
                selg,
            ],
            axis=1,
        )
    )
    shared = {"wall": wall, "sm": sm, "selb": selb}
    in_maps = []
    for core in range(NCORES):
        b, h = core // 2, core % 2
        m = dict(shared)
        m["xq"] = np.ascontiguousarray(
            x[b][:, h * LQ : (h + 1) * LQ].reshape(2, 128, LQ)
        )
        m["xo"] = np.ascontiguousarray(
            x[b][:, (1 - h) * LQ : (2 - h) * LQ].reshape(2, 128, LQ)
        )
        in_maps.append(m)
    return in_maps


def kernel(**inputs) -> np.ndarray:
    from concourse.bass_utils import run_bass_kernel_spmd

    if "nc" not in _CACHE:
        _CACHE["nc"] = _build_program()
    nc = _CACHE["nc"]

    in_maps = _host_inputs(**inputs)
    res = run_bass_kernel_spmd(nc, in_maps, list(range(NCORES)))

    out = np.empty((B, C, L), dtype=np.float32)
    for core in range(NCORES):
        b, h = core // 2, core % 2
        out[b][:, h * LQ : (h + 1) * LQ] = res.results[core]["y"].reshape(C, LQ)
    return out
